# revision 4
# baseline (speedup 1.0000x reference)
"""Trainium2 Bass kernel for nn_CriterionPairWiseforWholeFeatAfterPool.

Computation (reference): select feat_ind slice -> MaxPool2d with kernel
(H/2, W/2) producing a 2x2 pooled map per (sample, channel) -> L2-normalize
over channels -> per-sample 4x4 gram over the pooled spatial positions ->
scalar MSE-style loss between teacher/student grams.

Strategy (data-parallel per the sharding hint): shard the batch axis B=16
across 8 NeuronCores (2 samples/core, 64 MiB/core of HBM->SBUF traffic).
Channels ride the 128 SBUF partitions; every 64x64 max-pool window reduces
on the vector engine (free-axis reduce_max over strided quadrant views);
partial-max columns fold on host in the tiny gram/loss epilogue.

Engine-balance design (the core trick): on each even-numbered physical
NeuronCore one *edge* SDMA engine (idx 0 or idx 15) runs ~23% slower
(pair-shared AXI port contention), which serializes the whole stream
behind it when all 16 engines carry equal bytes.  Measured engine
assignment on this silicon:
  - HWDGE (sync/scalar ring): engine = descriptor position % 16,
    reset to 0 each op -> per-op descriptor COUNT controls engine load;
    n % 16 == 0 gives a perfectly flat 16-way split.
  - SWDGE (gpsimd ring):      engine = (partition % 8) + 8*(partition>=64)
    -> partition choice controls engine load exactly.
So each (sample, tensor, channel-block) region of 128 rows streams as:
  - H1a/H1b (sync, flat): rows 0:48 / 48:96 for all 128 partitions,
    128 descriptors each -> 8 descs/engine, every engine equal.
  - S1/S2 (gpsimd): rows 96:128 for the 112 NON-victim partitions
    (p%8!=0 below 64, p%8!=7 above) -> engines 1-14 only, +16 KiB each.
  - H2A/H2B (sync, flat): the 16 victim partitions' rows 96:128,
    relocated as 32 eight-row pieces onto partitions 0..63 at a spare
    column; 32 descriptors -> 2 descs/engine, every engine equal.
Per-region engine load: victims 400 KiB, middles 528 KiB (ratio 0.76,
matching the measured ~0.77 slow/fast engine rate ratio), so a slow edge
engine finishes no later than the healthy middle engines.  Host folds the
extra partial-max columns (48/16/32-row windows and relocated victim
tails) before the gram.
"""

import contextlib

import numpy as np

import concourse.bacc as bacc
import concourse.mybir as mybir
from concourse.bass_utils import run_bass_kernel_spmd

N_CORES = 8
P = 128           # SBUF partitions
B_LOC = 2         # samples per core (16 / 8)
C = 256           # channels
H = 128
W = 128
N_REG = B_LOC * 2 * (C // P)   # 8 regions/core: (sample, S|T, channel-block)
N_COLS = 10 * N_REG            # pooled partial-max column pairs
NBUF = 2                       # region slots (2 x 68 KiB/partition)

# slot column layout (f32 elems per partition)
COL_H1 = 0          # rows 0:96 of own channel            [0, 12288)
COL_S = 12288       # rows 96:128 of own channel (middles) [12288, 16384)
COL_H2 = 16384      # relocated victim tails, parts 0..63  [16384, 17408)
SLOT = 17408

_NC = None


def _build_nc():
    """Build + compile the per-core SPMD Bass program (same NEFF on all cores)."""
    nc = bacc.Bacc("TRN2", target_bir_lowering=False, debug=False,
                   num_devices=N_CORES)
    s = nc.dram_tensor("s", [B_LOC, C, H, W], mybir.dt.float32,
                       kind="ExternalInput").ap()
    t = nc.dram_tensor("t", [B_LOC, C, H, W], mybir.dt.float32,
                       kind="ExternalInput").ap()
    out = nc.dram_tensor("pooled", [P, N_COLS], mybir.dt.float32,
                         kind="ExternalOutput").ap()

    # region list: (b, x, cb) in stream order
    regions = [(b, x, cb)
               for b in range(B_LOC) for x in range(2) for cb in range(2)]

    def views(r):
        b, x, cb = regions[r]
        x2 = (s, t)[x][b, cb * P:(cb + 1) * P]        # [128, H, W]
        ab = x2.rearrange("(a bb) h w -> a bb h w", bb=8)  # a=16 blocks of 8
        return x2, ab

    with contextlib.ExitStack() as ctx:
        bufs = [ctx.enter_context(
            nc.sbuf_tensor(f"buf{i}", [P, SLOT], mybir.dt.float32))
            for i in range(NBUF)]
        pooled = ctx.enter_context(
            nc.sbuf_tensor("pooled_sb", [P, N_COLS], mybir.dt.float32))
        # per-(slot, op-kind) semaphores; the h2/s sems take increments from
        # 2 resp. 8 concurrent DMAs per region (atomic incs; slot reuse is
        # serialized through red_sem)
        names = ["h2", "s", "h1a", "h1b"]
        sems = {(i, n): ctx.enter_context(nc.semaphore(f"dma_{n}{i}"))
                for i in range(NBUF) for n in names}
        out_sem = ctx.enter_context(nc.semaphore("out_sem"))
        red_sem = ctx.enter_context(nc.semaphore("red_sem"))
        block = ctx.enter_context(nc.Block())

        R_PER = 5  # reduces per region

        @block.sync
        def _(sync):
            for r in range(N_REG):
                sl = r % NBUF
                buf = bufs[sl]
                x2, ab = views(r)
                if r >= NBUF:
                    sync.wait_ge(red_sem, R_PER * (r - NBUF + 1))
                # H2A/H2B: victim tails relocated as 8-row pieces onto
                # parts 0..63; 32 descriptors each -> flat 2/engine.
                # lower victims: channels 8a (a=0..7); pieces (a,k): rows
                # 96+8k:104+8k -> dst partition 4a+k (linear pairing).
                srcA = ab[0:8, 0:1, 96:128, :].rearrange(
                    "a bb (k h) w -> (a bb) k (h w)", k=4)
                sync.dma_start(
                    buf[0:32, COL_H2:COL_H2 + 1024], srcA
                ).then_inc(sems[(sl, "h2")], 16)
                # upper victims: channels 8a+7 (a=8..15) -> parts 32..63
                srcB = ab[8:16, 7:8, 96:128, :].rearrange(
                    "a bb (k h) w -> (a bb) k (h w)", k=4)
                sync.dma_start(
                    buf[32:64, COL_H2:COL_H2 + 1024], srcB
                ).then_inc(sems[(sl, "h2")], 16)
                # H1a/H1b: rows 0:48 and 48:96, all 128 partitions, flat.
                sync.dma_start(
                    buf[:, 0:6144],
                    x2[:, 0:48, :].rearrange("c h w -> c (h w)")
                ).then_inc(sems[(sl, "h1a")], 16)
                sync.dma_start(
                    buf[:, 6144:12288],
                    x2[:, 48:96, :].rearrange("c h w -> c (h w)")
                ).then_inc(sems[(sl, "h1b")], 16)
            sync.wait_ge(red_sem, R_PER * N_REG)
            sync.dma_start(out, pooled[:, :]).then_inc(out_sem, 16)
            sync.wait_ge(out_sem, 16)

        @block.gpsimd
        def _(gpsimd):
            for r in range(N_REG):
                sl = r % NBUF
                buf = bufs[sl]
                x2, _ab = views(r)
                if r >= NBUF:
                    gpsimd.wait_ge(red_sem, R_PER * (r - NBUF + 1))
                # S ops: rows 96:128 for non-victim partitions, one op per
                # p%8 residue (strided partition slice).  SWDGE maps
                # partition p -> engine (p%8) + 8*(p>=64), so these ops
                # land exactly on engines 1..14, never on e0/e15.
                for bb in range(8):
                    lo = bb if bb != 0 else bb + 64      # skip lower victims
                    hi = 128 if bb != 7 else 64          # skip upper victims
                    gpsimd.dma_start(
                        buf[lo:hi:8, COL_S:COL_S + 4096],
                        x2[lo:hi:8, 96:128, :].rearrange("c h w -> c (h w)")
                    ).then_inc(sems[(sl, "s")], 16)

        @block.vector
        def _(vector):
            def red(dst_cols, view):
                vector.tensor_reduce(
                    dst_cols, view, axis=mybir.AxisListType.XY,
                    op=mybir.AluOpType.max).then_inc(red_sem, 1)

            for r in range(N_REG):
                sl = r % NBUF
                buf = bufs[sl]
                u = r // NBUF + 1
                c0 = 10 * r
                # H2 tails (arrive first: small): 8-row partials, parts 0:64
                vector.wait_ge(sems[(sl, "h2")], 32 * u)
                red(pooled[0:64, c0 + 6:c0 + 8],
                    buf[0:64, COL_H2:COL_H2 + 1024].rearrange(
                        "p (h j w) -> p j h w", j=2, w=64))
                # S tails: rows 96:128 partials (victim partitions stale,
                # host ignores them)
                vector.wait_ge(sems[(sl, "s")], 128 * u)
                red(pooled[:, c0 + 4:c0 + 6],
                    buf[:, COL_S:COL_S + 4096].rearrange(
                        "p (h j w) -> p j h w", j=2, w=64))
                # H1a: rows 0:48 partial of band 0
                vector.wait_ge(sems[(sl, "h1a")], 16 * u)
                red(pooled[:, c0 + 0:c0 + 2],
                    buf[:, 0:6144].rearrange(
                        "p (h j w) -> p j h w", j=2, w=64))
                # H1b: rows 48:64 (band-0 partial) + rows 64:96 (band-1 p1)
                vector.wait_ge(sems[(sl, "h1b")], 16 * u)
                red(pooled[:, c0 + 2:c0 + 4],
                    buf[:, 6144:8192].rearrange(
                        "p (h j w) -> p j h w", j=2, w=64))
                red(pooled[:, c0 + 8:c0 + 10],
                    buf[:, 8192:12288].rearrange(
                        "p (h j w) -> p j h w", j=2, w=64))

    nc.compile()
    return nc


def get_nc():
    global _NC
    if _NC is None:
        _NC = _build_nc()
    return _NC


def make_in_maps(fS, fT):
    """Per-core input dicts: batch-sharded contiguous slices."""
    return [{"s": np.ascontiguousarray(fS[B_LOC * i:B_LOC * (i + 1)]),
             "t": np.ascontiguousarray(fT[B_LOC * i:B_LOC * (i + 1)])}
            for i in range(N_CORES)]


def finish(pooled_list):
    """Host epilogue: fold partial maxes, gram + normalize + loss."""
    B = B_LOC * N_CORES
    fS = np.full((B, C, 4), -np.inf)
    fT = np.full((B, C, 4), -np.inf)
    regions = [(b, x, cb)
               for b in range(B_LOC) for x in range(2) for cb in range(2)]
    lower_vic = [8 * a for a in range(8)]           # p%8==0, p<64
    upper_vic = [8 * a + 7 for a in range(8, 16)]   # p%8==7, p>=64
    victims = set(lower_vic + upper_vic)
    for i, arr in enumerate(pooled_list):
        a = np.asarray(arr)  # [P, N_COLS]
        for r, (bl, xi, cb) in enumerate(regions):
            f = (fS, fT)[xi]
            bi = i * B_LOC + bl
            c0 = 10 * r
            ch = slice(cb * P, (cb + 1) * P)
            tgt = f[bi, ch]  # [128, 4]
            # band 0 (cols 0,1): partials rows 0:48 and 48:64
            np.maximum(tgt[:, 0:2], a[:, c0:c0 + 2], out=tgt[:, 0:2])
            np.maximum(tgt[:, 0:2], a[:, c0 + 2:c0 + 4], out=tgt[:, 0:2])
            # band 1 (cols 2,3): rows 64:96 partial (all channels)
            np.maximum(tgt[:, 2:4], a[:, c0 + 8:c0 + 10], out=tgt[:, 2:4])
            # rows 96:128: middles from S cols; victims from H2 cols
            for p in range(P):
                if p not in victims:
                    np.maximum(tgt[p, 2:4], a[p, c0 + 4:c0 + 6],
                               out=tgt[p, 2:4])
            for ai, p in enumerate(lower_vic):      # dst part 4a+k
                for k in range(4):
                    np.maximum(tgt[p, 2:4], a[4 * ai + k, c0 + 6:c0 + 8],
                               out=tgt[p, 2:4])
            for ai, p in enumerate(upper_vic):      # dst part 32+4(a-8)+k
                for k in range(4):
                    np.maximum(tgt[p, 2:4],
                               a[32 + 4 * ai + k, c0 + 6:c0 + 8],
                               out=tgt[p, 2:4])

    def sim(f):
        G = np.einsum('bcm,bcn->bmn', f, f)
        d = np.sqrt(np.einsum('bmm->bm', G)) + 1e-8
        return G / (d[:, :, None] * d[:, None, :])

    loss = ((sim(fT) - sim(fS)) ** 2).sum() / (4 * 4) / B
    return np.float32(loss)


def run_device(fS, fT, **spmd_kwargs):
    """Run the compiled program on the 8 cores; returns (pooled_list, results)."""
    res = run_bass_kernel_spmd(get_nc(), make_in_maps(fS, fT),
                               core_ids=list(range(N_CORES)), **spmd_kwargs)
    pooled_list = [res.results[i]["pooled"] for i in range(N_CORES)]
    return pooled_list, res


def kernel(preds_S, preds_T, feat_ind):
    fi = int(np.asarray(feat_ind))
    fS = np.ascontiguousarray(np.asarray(preds_S)[fi], dtype=np.float32)
    fT = np.ascontiguousarray(np.asarray(preds_T)[fi], dtype=np.float32)
    try:
        pooled_list, _ = run_device(fS, fT)
    except Exception:
        # one retry: a cold device occasionally reports a transient
        # NRT execution error on the very first NEFF launch
        pooled_list, _ = run_device(fS, fT)
    return finish(pooled_list)
